# revision 13
# baseline (speedup 1.0000x reference)
"""Trainium2 Bass kernel for nn_MeanPooling (segment_reduce).

Computes out[b,e,h] = (sum_l entity_mapping[b,e,l] * doc_state[b,l,h]) / entity_lens[b,e]
for B=16, E=128, L=2048, H=1024.

Sharding: data-parallel over batch B across 8 NeuronCores (2 batches per core).
Per core, each batch is a (E=128, L=2048) @ (L=2048, H=1024) matmul.

Precision strategy (gate is rel_err < 2e-2; this measures ~7e-4):
  - doc_state is cast to fp16 on the host: halves HBM traffic AND runs the
    PE at 1 cycle/row (vs 4 for fp32). fp32 PSUM accumulation.
  - entity_mapping is binary, so it is EXACT in fp8e4 (quarter traffic); it
    is transposed to [P, KT*E] on the host so no on-device transposes are
    needed. The matmul runs mixed fp8 weights x fp16 moving.
  - The device stores the raw fp16 sums (|sum| < ~80, far from fp16 range);
    the host divides by entity_lens after the gather. No lens DMA, no
    reciprocal, no scaled eviction.

Schedule per core (DMA-bound: ~9 MB at ~360 GB/s):
  - doc is laid out partition-major on the host ([b, p, ko, h]) so every
    chunk descriptor is w*2KB contiguous per partition; chunks stream
    in-order on the Sync ring (order inversion across rings stalls the PE).
    Map pieces ride the Scalar ring; output stores ride the GpSimd ring.
  - Chunk plan 1,2,3,4,3,2,1 k-tiles: small ends cut pipeline fill/drain
    latency, big middle cuts trigger/semaphore overhead.
  - Per batch: 16 k-tiles x 2 psum groups of 512 cols, k-major. On the last
    k-tile group 1 is computed first so its eviction overlaps group 0's
    final matmul. Evictions are 256-col halves split across the ACT and
    DVE engines; the final store is split in two so the first half
    transfers while the second half evicts.
"""

import os

import numpy as np

B, E, L, H = 16, 128, 2048, 1024
N_CORES = 8
B_PER_CORE = B // N_CORES
P = 128
KT = L // P  # 16 k-tiles per batch
GW = 512  # psum group width (one PSUM bank of fp32)
NG = H // GW

# doc DMA chunk plan (k-tiles per DMA), per batch
_plan = os.environ.get("BASS_DOC_PLAN", "1,2,3,4,3,2,1")
DOC_PLAN = [int(x) for x in _plan.split(",")]
assert sum(DOC_PLAN) == KT
# map piece plans (k-tiles per DMA) per batch
_mplan0 = os.environ.get("BASS_MAP_PLAN0", "4,12")
MAP_PLAN0 = [int(x) for x in _mplan0.split(",")]
assert sum(MAP_PLAN0) == KT
_mplan1 = os.environ.get("BASS_MAP_PLAN1", "4,12")
MAP_PLAN1 = [int(x) for x in _mplan1.split(",")]
assert sum(MAP_PLAN1) == KT

# f8mm: fp8 map fed straight to the PE as weights (mixed with fp16 moving)
# f8cast: fp8 map DMA, DVE-cast to fp16 before the PE
# f16: fp16 map DMA
MAP_MODE = os.environ.get("BASS_MAP_MODE", "f8mm")


def _map_np_dt():
    if MAP_MODE in ("f8mm", "f8cast"):
        import ml_dtypes

        return ml_dtypes.float8_e4m3fn
    return np.float16


_CACHE = {}


def _build_bass():
    import concourse.mybir as mybir
    from concourse import bacc
    from concourse.bass import ds as bass_ds, ts
    from concourse.tile import TileContext

    f32 = mybir.dt.float32
    f16 = mybir.dt.float16
    map_dt = mybir.dt.float8e4 if MAP_MODE in ("f8mm", "f8cast") else f16

    nc = bacc.Bacc(None, target_bir_lowering=False)
    # doc[b][p][ko*H + h] = doc_state[b, ko*P + p, h]   (partition-major)
    doc = nc.dram_tensor("doc_state", [B_PER_CORE, P, KT * H], f16, kind="ExternalInput")
    # mapt[b][p][ko*E + e] = entity_mapping[b, e, ko*P + p]   (binary)
    mp = nc.dram_tensor("mapt", [B_PER_CORE, P, KT * E], map_dt, kind="ExternalInput")
    # raw sums; host divides by entity_lens
    out = nc.dram_tensor("out", [B_PER_CORE, E, H], f16, kind="ExternalOutput")

    with TileContext(nc) as tc:
        with (
            tc.tile_pool(name="mapp", bufs=2 * (len(MAP_PLAN0) + len(MAP_PLAN1))) as map_pool,
            tc.tile_pool(name="doc", bufs=2 * len(DOC_PLAN)) as doc_pool,
            tc.tile_pool(name="outp", bufs=2) as out_pool,
            tc.tile_pool(name="psum", bufs=2 * NG, space="PSUM") as psum_pool,
        ):
            # ---- Phase 1: all input DMAs, in consumption order ----
            map_tiles = [[], []]  # [b] -> [(k0, w, tile)]
            doc_tiles = [[None] * KT for _ in range(B_PER_CORE)]  # [b][k] -> (tile, kk)

            def load_map(b, k0, w):
                mt = map_pool.tile([P, w * E], map_dt, tag="map_sb", name="map_sb")
                nc.scalar.dma_start(out=mt, in_=mp[b][:, bass_ds(k0 * E, w * E)])
                if MAP_MODE == "f8cast":
                    mt16 = map_pool.tile([P, w * E], f16, tag="map16", name="map16")
                    nc.vector.tensor_copy(mt16, mt)
                    mt = mt16
                map_tiles[b].append((k0, w, mt))

            def load_doc(b, k0, w):
                dtile = doc_pool.tile(
                    [P, max(DOC_PLAN), H], f16, tag="dtile", name="dtile"
                )[:, :w, :]
                nc.sync.dma_start(out=dtile, in_=doc[b][:, bass_ds(k0 * H, w * H)])
                for kk in range(w):
                    doc_tiles[b][k0 + kk] = (dtile, kk)

            doc_chunks = []
            k0 = 0
            for w in DOC_PLAN:
                doc_chunks.append((k0, w))
                k0 += w

            load_map(0, 0, MAP_PLAN0[0])
            load_doc(0, *doc_chunks[0])
            kk0 = MAP_PLAN0[0]
            for w in MAP_PLAN0[1:]:
                load_map(0, kk0, w)
                kk0 += w
            for c in doc_chunks[1:-2]:
                load_doc(0, *c)
            # batch-1 map pieces arrive before batch-0's tail chunks
            load_map(1, 0, MAP_PLAN1[0])
            for c in doc_chunks[-2:]:
                load_doc(0, *c)
            kk1 = MAP_PLAN1[0]
            for w in MAP_PLAN1[1:]:
                load_map(1, kk1, w)
                kk1 += w
            for c in doc_chunks:
                load_doc(1, *c)

            # ---- Phase 2: matmuls + eviction per batch ----
            def lhsT_for(b, k):
                for piece_k0, w, mt in map_tiles[b]:
                    if piece_k0 <= k < piece_k0 + w:
                        return mt[:, ts(k - piece_k0, E)]
                raise AssertionError(k)

            HW_ = GW // 2
            for b in range(B_PER_CORE):
                psums = [
                    psum_pool.tile([E, GW], f32, tag="ps", name="ps") for _ in range(NG)
                ]
                out_sb = out_pool.tile([E, H], f16, tag="out_sb", name="out_sb")
                for k in range(KT):
                    lhsT = lhsT_for(b, k)
                    dtile, kk = doc_tiles[b][k]
                    groups = range(NG) if k < KT - 1 else reversed(range(NG))
                    for g in groups:
                        nc.tensor.matmul(
                            psums[g],
                            lhsT=lhsT,
                            rhs=dtile[:, kk, ts(g, GW)],
                            start=(k == 0),
                            stop=(k == KT - 1),
                        )
                # evict psum -> fp16 SBUF (raw sums, pure dtype convert) in
                # 256-col halves on ACT + DVE. Group 1 closes first on the
                # last k-tile, so evict/store it first; the final (group 0)
                # store is split so its first half transfers early.
                for g in reversed(range(NG)):
                    nc.scalar.activation(
                        out_sb[:, bass_ds(g * GW, HW_)],
                        psums[g][:, bass_ds(0, HW_)],
                        mybir.ActivationFunctionType.Copy,
                    )
                    nc.vector.tensor_copy(
                        out_sb[:, bass_ds(g * GW + HW_, HW_)],
                        psums[g][:, bass_ds(HW_, HW_)],
                    )
                    if g == 0:
                        nc.sync.dma_start(
                            out=out[b][:, bass_ds(0, HW_)],
                            in_=out_sb[:, bass_ds(0, HW_)],
                        )
                        nc.gpsimd.dma_start(
                            out=out[b][:, bass_ds(HW_, HW_)],
                            in_=out_sb[:, bass_ds(HW_, HW_)],
                        )
                    else:
                        nc.scalar.dma_start(
                            out=out[b][:, ts(g, GW)], in_=out_sb[:, ts(g, GW)]
                        )

    nc.finalize()
    return nc


def _get_nc():
    if "nc" not in _CACHE:
        _CACHE["nc"] = _build_bass()
    return _CACHE["nc"]


def kernel(doc_state, entity_mapping, entity_lens, **run_kwargs):
    from concourse.bass_utils import run_bass_kernel_spmd

    nc = _get_nc()
    map_np_dt = _map_np_dt()
    in_maps = []
    for i in range(N_CORES):
        sl = slice(i * B_PER_CORE, (i + 1) * B_PER_CORE)
        doc16 = (
            np.ascontiguousarray(doc_state[sl], dtype=np.float32)
            .astype(np.float16)
            .reshape(B_PER_CORE, KT, P, H)
            .transpose(0, 2, 1, 3)  # (b, P, KT, H)
            .reshape(B_PER_CORE, P, KT * H)
        )
        mt = (
            np.ascontiguousarray(entity_mapping[sl], dtype=np.float32)
            .transpose(0, 2, 1)  # (b, L, E)
            .reshape(B_PER_CORE, KT, P, E)
            .transpose(0, 2, 1, 3)  # (b, P, KT, E)
            .reshape(B_PER_CORE, P, KT * E)
        )
        in_maps.append(
            {
                "doc_state": np.ascontiguousarray(doc16),
                "mapt": np.ascontiguousarray(mt).astype(map_np_dt),
            }
        )
    res = run_bass_kernel_spmd(nc, in_maps, core_ids=list(range(N_CORES)), **run_kwargs)
    sums = np.concatenate([r["out"] for r in res.results], axis=0).astype(np.float32)
    out = sums / entity_lens[:, :, None].astype(np.float32)
    if run_kwargs:
        _CACHE["last_result"] = res
    return out


# revision 14
# speedup vs baseline: 1.0094x; 1.0094x over previous
"""Trainium2 Bass kernel for nn_MeanPooling (segment_reduce).

Computes out[b,e,h] = (sum_l entity_mapping[b,e,l] * doc_state[b,l,h]) / entity_lens[b,e]
for B=16, E=128, L=2048, H=1024.

Sharding: data-parallel over batch B across 8 NeuronCores (2 batches per core).
Per core, each batch is a (E=128, L=2048) @ (L=2048, H=1024) matmul.

Precision strategy (gate is rel_err < 2e-2; this measures ~7e-4):
  - doc_state is cast to fp16 on the host: halves HBM traffic AND runs the
    PE at 1 cycle/row (vs 4 for fp32). fp32 PSUM accumulation.
  - entity_mapping is binary, so it is EXACT in fp8e4 (quarter traffic); it
    is transposed to [P, KT*E] on the host so no on-device transposes are
    needed. The matmul runs mixed fp8 weights x fp16 moving.
  - The device stores the raw fp16 sums (|sum| < ~80, far from fp16 range);
    the host divides by entity_lens after the gather. No lens DMA, no
    reciprocal, no scaled eviction.

Schedule per core (DMA-bound: ~9 MB at ~360 GB/s):
  - doc is laid out partition-major on the host ([b, p, ko, h]) so every
    chunk descriptor is w*2KB contiguous per partition; chunks stream
    in-order on the Sync ring (order inversion across rings stalls the PE).
    Map pieces ride the Scalar ring; output stores ride the GpSimd ring.
  - Chunk plan 1,2,3,4,3,2,1 k-tiles: small ends cut pipeline fill/drain
    latency, big middle cuts trigger/semaphore overhead.
  - Per batch: 16 k-tiles x 2 psum groups of 512 cols, k-major. On the last
    k-tile group 1 is computed first so its eviction overlaps group 0's
    final matmul. Evictions are 256-col halves split across the ACT and
    DVE engines; the final store is split in two so the first half
    transfers while the second half evicts.
"""

import os

import numpy as np

B, E, L, H = 16, 128, 2048, 1024
N_CORES = 8
B_PER_CORE = B // N_CORES
P = 128
KT = L // P  # 16 k-tiles per batch
GW = 512  # psum group width (one PSUM bank of fp32)
NG = H // GW

# doc DMA chunk plan (k-tiles per DMA), per batch
_plan = os.environ.get("BASS_DOC_PLAN", "2,4,4,4,1,1")
DOC_PLAN = [int(x) for x in _plan.split(",")]
assert sum(DOC_PLAN) == KT
# map piece plans (k-tiles per DMA) per batch
_mplan0 = os.environ.get("BASS_MAP_PLAN0", "16")
MAP_PLAN0 = [int(x) for x in _mplan0.split(",")]
assert sum(MAP_PLAN0) == KT
_mplan1 = os.environ.get("BASS_MAP_PLAN1", "16")
MAP_PLAN1 = [int(x) for x in _mplan1.split(",")]
assert sum(MAP_PLAN1) == KT

# f8mm: fp8 map fed straight to the PE as weights (mixed with fp16 moving)
# f8cast: fp8 map DMA, DVE-cast to fp16 before the PE
# f16: fp16 map DMA
MAP_MODE = os.environ.get("BASS_MAP_MODE", "f8mm")


def _map_np_dt():
    if MAP_MODE in ("f8mm", "f8cast"):
        import ml_dtypes

        return ml_dtypes.float8_e4m3fn
    return np.float16


_CACHE = {}


def _build_bass():
    import concourse.mybir as mybir
    from concourse import bacc
    from concourse.bass import ds as bass_ds, ts
    from concourse.tile import TileContext

    f32 = mybir.dt.float32
    f16 = mybir.dt.float16
    map_dt = mybir.dt.float8e4 if MAP_MODE in ("f8mm", "f8cast") else f16

    nc = bacc.Bacc(None, target_bir_lowering=False)
    # doc[b][p][ko*H + h] = doc_state[b, ko*P + p, h]   (partition-major)
    doc = nc.dram_tensor("doc_state", [B_PER_CORE, P, KT * H], f16, kind="ExternalInput")
    # mapt[b][p][ko*E + e] = entity_mapping[b, e, ko*P + p]   (binary)
    mp = nc.dram_tensor("mapt", [B_PER_CORE, P, KT * E], map_dt, kind="ExternalInput")
    # raw sums; host divides by entity_lens
    out = nc.dram_tensor("out", [B_PER_CORE, E, H], f16, kind="ExternalOutput")

    NMAP = len(MAP_PLAN0) + len(MAP_PLAN1)
    NDOC = 2 * len(DOC_PLAN)
    with TileContext(nc) as tc:
        with (
            tc.tile_pool(name="sbuf", bufs=1) as sbuf_pool,
            tc.tile_pool(name="psum", bufs=2 * NG, space="PSUM") as psum_pool,
        ):
            # ---- Phase 1: all input DMAs, in consumption order ----
            map_tiles = [[], []]  # [b] -> [(k0, w, tile)]
            doc_tiles = [[None] * KT for _ in range(B_PER_CORE)]  # [b][k] -> (tile, kk)

            def load_map(b, k0, w):
                mt = sbuf_pool.tile(
                    [P, w * E], map_dt, tag="map_sb", name="map_sb", bufs=NMAP
                )
                nc.scalar.dma_start(out=mt, in_=mp[b][:, bass_ds(k0 * E, w * E)])
                if MAP_MODE == "f8cast":
                    mt16 = sbuf_pool.tile(
                        [P, w * E], f16, tag="map16", name="map16", bufs=NMAP
                    )
                    nc.vector.tensor_copy(mt16, mt)
                    mt = mt16
                map_tiles[b].append((k0, w, mt))

            def load_doc(b, k0, w):
                dtile = sbuf_pool.tile(
                    [P, max(DOC_PLAN), H], f16, tag="dtile", name="dtile", bufs=NDOC
                )[:, :w, :]
                nc.sync.dma_start(out=dtile, in_=doc[b][:, bass_ds(k0 * H, w * H)])
                for kk in range(w):
                    doc_tiles[b][k0 + kk] = (dtile, kk)

            doc_chunks = []
            k0 = 0
            for w in DOC_PLAN:
                doc_chunks.append((k0, w))
                k0 += w

            load_map(0, 0, MAP_PLAN0[0])
            load_doc(0, *doc_chunks[0])
            kk0 = MAP_PLAN0[0]
            for w in MAP_PLAN0[1:]:
                load_map(0, kk0, w)
                kk0 += w
            for c in doc_chunks[1:-2]:
                load_doc(0, *c)
            # batch-1 map pieces arrive before batch-0's tail chunks
            load_map(1, 0, MAP_PLAN1[0])
            for c in doc_chunks[-2:]:
                load_doc(0, *c)
            kk1 = MAP_PLAN1[0]
            for w in MAP_PLAN1[1:]:
                load_map(1, kk1, w)
                kk1 += w
            for c in doc_chunks:
                load_doc(1, *c)

            # ---- Phase 2: matmuls + eviction per batch ----
            def lhsT_for(b, k):
                for piece_k0, w, mt in map_tiles[b]:
                    if piece_k0 <= k < piece_k0 + w:
                        return mt[:, ts(k - piece_k0, E)]
                raise AssertionError(k)

            HW_ = GW // 2
            for b in range(B_PER_CORE):
                psums = [
                    psum_pool.tile([E, GW], f32, tag="ps", name="ps") for _ in range(NG)
                ]
                out_sb = sbuf_pool.tile(
                    [E, H], f16, tag="out_sb", name="out_sb", bufs=2
                )
                for k in range(KT):
                    lhsT = lhsT_for(b, k)
                    dtile, kk = doc_tiles[b][k]
                    groups = range(NG) if k < KT - 1 else reversed(range(NG))
                    for g in groups:
                        nc.tensor.matmul(
                            psums[g],
                            lhsT=lhsT,
                            rhs=dtile[:, kk, ts(g, GW)],
                            start=(k == 0),
                            stop=(k == KT - 1),
                        )
                # evict psum -> fp16 SBUF (raw sums, pure dtype convert) in
                # 256-col halves on ACT + DVE. Group 1 closes first on the
                # last k-tile, so evict/store it first; the final (group 0)
                # store is split so its first half transfers early.
                for g in reversed(range(NG)):
                    nc.scalar.activation(
                        out_sb[:, bass_ds(g * GW, HW_)],
                        psums[g][:, bass_ds(0, HW_)],
                        mybir.ActivationFunctionType.Copy,
                    )
                    nc.vector.tensor_copy(
                        out_sb[:, bass_ds(g * GW + HW_, HW_)],
                        psums[g][:, bass_ds(HW_, HW_)],
                    )
                    if g == 0:
                        nc.sync.dma_start(
                            out=out[b][:, bass_ds(0, HW_)],
                            in_=out_sb[:, bass_ds(0, HW_)],
                        )
                        nc.gpsimd.dma_start(
                            out=out[b][:, bass_ds(HW_, HW_)],
                            in_=out_sb[:, bass_ds(HW_, HW_)],
                        )
                    else:
                        nc.scalar.dma_start(
                            out=out[b][:, ts(g, GW)], in_=out_sb[:, ts(g, GW)]
                        )

    nc.finalize()
    return nc


def _get_nc():
    if "nc" not in _CACHE:
        _CACHE["nc"] = _build_bass()
    return _CACHE["nc"]


def kernel(doc_state, entity_mapping, entity_lens, **run_kwargs):
    from concourse.bass_utils import run_bass_kernel_spmd

    nc = _get_nc()
    map_np_dt = _map_np_dt()
    in_maps = []
    for i in range(N_CORES):
        sl = slice(i * B_PER_CORE, (i + 1) * B_PER_CORE)
        doc16 = (
            np.ascontiguousarray(doc_state[sl], dtype=np.float32)
            .astype(np.float16)
            .reshape(B_PER_CORE, KT, P, H)
            .transpose(0, 2, 1, 3)  # (b, P, KT, H)
            .reshape(B_PER_CORE, P, KT * H)
        )
        mt = (
            np.ascontiguousarray(entity_mapping[sl], dtype=np.float32)
            .transpose(0, 2, 1)  # (b, L, E)
            .reshape(B_PER_CORE, KT, P, E)
            .transpose(0, 2, 1, 3)  # (b, P, KT, E)
            .reshape(B_PER_CORE, P, KT * E)
        )
        in_maps.append(
            {
                "doc_state": np.ascontiguousarray(doc16),
                "mapt": np.ascontiguousarray(mt).astype(map_np_dt),
            }
        )
    res = run_bass_kernel_spmd(nc, in_maps, core_ids=list(range(N_CORES)), **run_kwargs)
    sums = np.concatenate([r["out"] for r in res.results], axis=0).astype(np.float32)
    out = sums / entity_lens[:, :, None].astype(np.float32)
    if run_kwargs:
        _CACHE["last_result"] = res
    return out
